# revision 18
# baseline (speedup 1.0000x reference)
"""Chamfer loss Bass/Tile kernel for Trainium2 (8 NeuronCores, SPMD).

Problem: x, y [B=32, D=128, N=2048] f32, mask [B, N] bool (shared by x and y).
  d[b,i,j] = ||x_i - y_j||^2;  loss = mean_b( sum_j min_i d + sum_i min_j d )
  (mins/sums over valid entries only).

Strategy (v5):
  - ONE fp8 (e4m3) DoubleRow matmul per [128 x L] tile computes
      W = x.y - x2/2 - y2/2 - 480*(1-m_i) - 480*(1-m_j)  (= -d/2, biased)
    directly in PSUM: the DoubleRow second k-tile carries 8 augmented
    contraction rows encoding the norms (3-term fp8 residual splits, ~2e-4
    relative) and the mask penalties. 0.5 PE cycles/output element, no
    prefill, no downstream bias work.
  - Crop: mask is a prefix; only W[i<L, j<L] can matter, with L =
    ceil(last_set_bit/128)*128. Batches sorted by len across cores so the 4
    per-core slots share compile-time crops (one NEFF, SPMD).
  - Every chunk [128, L] of PSUM is consumed by a SPLIT evacuation, halving
    PSUM residency (the pipeline pacer) and balancing the engines:
      cols [0, WA):  ACT Exp-evacuates -> exp(W+22) bf16, its accumulator
        emits the row sum (softmin/LSE at beta=1 on the d/2 scale).
      cols [WA, L):  Pool tensor_scalar evacuates W bf16 with an exact
        rowmax accumulator.
    Both land in ONE U tile (exp-space left, raw right; columns are
    consistent across chunks). One full-width running tensor_tensor max
    per chunk builds the colmax chain: DVE mostly, Pool for a few chunks
    (two independent chains, combined on the host).
  - Device ships the two accumulator panels [128, nic] and the chain tiles
    [128, L] bf16; host does partition-maxes, logs, masks, -2/B (tiny numpy).
"""

import numpy as np
import ml_dtypes
from contextlib import ExitStack

import concourse.mybir as mybir
import concourse.tile as tile
from concourse import bacc

F32 = mybir.dt.float32
BF16 = mybir.dt.bfloat16
FP8 = mybir.dt.float8e4
AX = mybir.AxisListType
OP = mybir.AluOpType
ACTF = mybir.ActivationFunctionType
PM = mybir.MatmulPerfMode

B, D, N = 32, 128, 2048
CORES = 8
BPC = B // CORES          # batch slots per core
ICH = 128                 # i-chunk size (PSUM partition dim)
MASKPEN = 240.0           # TRN fp8e4m3 max normal; paired with a +/-2 partner
NP_FP8 = ml_dtypes.float8_e4m3   # concourse dt.py maps float8e4 to this
LSE_BIAS = 22.0           # global exp shift: exp(W + 22) spans ~[1e-33, 3e33]

WA_FRAC = 0.46            # ACT (exp) share of each chunk's columns
POOL_COL_FRAC = 0.11      # fraction of chunks whose colmax chain runs on Pool


def slot_geometry(L):
    """(wA, pool_col): ACT column split and per-chunk pool-colmax flags."""
    nic = L // ICH
    wA = int(round(WA_FRAC * L / 16)) * 16
    wA = min(max(wA, 16), L - 16)
    k = max(0, round(POOL_COL_FRAC * nic))
    pool_col = [False] * nic
    step = nic / k if k else 0
    for t in range(k):
        pool_col[min(nic - 1, int(t * step + step / 2))] = True
    return wA, pool_col


def build_nc(crops):
    nc = bacc.Bacc("TRN2", target_bir_lowering=False, debug=False)
    xa_d, ya_d, ra_d, rb_d, rd_d, rp_d = [], [], [], [], [], []
    for s, L in enumerate(crops):
        nic = L // ICH
        xa_d.append(nc.dram_tensor(f"xa{s}", [D, 2, L], FP8, kind="ExternalInput").ap())
        ya_d.append(nc.dram_tensor(f"ya{s}", [D, 2, L], FP8, kind="ExternalInput").ap())
        ra_d.append(nc.dram_tensor(f"ra{s}", [D, nic], F32, kind="ExternalOutput").ap())
        rb_d.append(nc.dram_tensor(f"rb{s}", [D, nic], F32, kind="ExternalOutput").ap())
        rd_d.append(nc.dram_tensor(f"rd{s}", [D, L], BF16, kind="ExternalOutput").ap())
        rp_d.append(nc.dram_tensor(f"rp{s}", [D, L], BF16, kind="ExternalOutput").ap())

    with tile.TileContext(nc) as tc:
        with ExitStack() as ctx:
            _emit(ctx, tc, crops, xa_d, ya_d, ra_d, rb_d, rd_d, rp_d)
    nc.compile()
    return nc


def _emit(ctx, tc, crops, xa_d, ya_d, ra_d, rb_d, rd_d, rp_d):
    nc = tc.nc
    io = ctx.enter_context(tc.tile_pool(name="io", bufs=2))
    up = ctx.enter_context(tc.tile_pool(name="up", bufs=4))
    rp_pool = ctx.enter_context(tc.tile_pool(name="rpool", bufs=2))
    small = ctx.enter_context(tc.tile_pool(name="small", bufs=2))
    pp = ctx.enter_context(tc.tile_pool(name="pp", bufs=2, space="PSUM"))
    pre = ctx.enter_context(tc.tile_pool(name="pre", bufs=1))

    lse_bias = pre.tile([D, 1], F32, tag="lse_bias")
    nc.gpsimd.memset(lse_bias[:], LSE_BIAS)
    # warm the Exp activation table before the first real chunk
    warm = pre.tile([D, 1], BF16, tag="warm")
    nc.scalar.activation(warm[:], lse_bias[:], ACTF.Exp, bias=0.0, scale=0.0)

    for s, L in enumerate(crops):
        nic = L // ICH
        wA, pool_col = slot_geometry(L)

        xa = io.tile([D, 2, L], FP8, tag="xa", name=f"xa{s}")
        ya = io.tile([D, 2, L], FP8, tag="ya", name=f"ya{s}")
        nc.sync.dma_start(out=ya[:, :, :512], in_=ya_d[s][:, :, :512])
        nc.sync.dma_start(out=xa[:, :, :ICH], in_=xa_d[s][:, :, :ICH])
        if L > 512:
            nc.sync.dma_start(out=ya[:, :, 512:], in_=ya_d[s][:, :, 512:])
        nc.sync.dma_start(out=xa[:, :, ICH:], in_=xa_d[s][:, :, ICH:])

        Rd = rp_pool.tile([D, L], BF16, tag="Rd", name=f"Rd{s}")
        Rp = rp_pool.tile([D, L], BF16, tag="Rp", name=f"Rp{s}")
        rsa = small.tile([D, nic], F32, tag="rsa", name=f"rsa{s}")  # sumexp
        rmb = small.tile([D, nic], F32, tag="rmb", name=f"rmb{s}")  # raw max
        first = {"d": True, "p": True}
        have_p = any(pool_col)

        for ic in range(nic):
            ps = pp.tile([D, L], F32, tag="ps")
            for j0 in range(0, L, 512):
                jw = min(512, L - j0)
                nc.tensor.matmul(
                    ps[:, j0:j0 + jw],
                    lhsT=xa[:, :, ic * ICH:(ic + 1) * ICH],
                    rhs=ya[:, :, j0:j0 + jw],
                    start=True, stop=True,
                    perf_mode=PM.DoubleRow)

            U = up.tile([D, L], BF16, tag="u")
            # split evacuation: exp-left on ACT (LSE row sums), raw-right on
            # Pool (exact row maxes); PSUM freed when both halves are read
            nc.scalar.activation(U[:, :wA], ps[:, :wA], ACTF.Exp,
                                 bias=lse_bias[:], scale=1.0,
                                 accum_out=rsa[:, ic:ic + 1])
            nc.gpsimd.tensor_scalar(U[:, wA:], ps[:, wA:], 0.0, None,
                                    op0=OP.add, op1=OP.max,
                                    accum_out=rmb[:, ic:ic + 1])
            if pool_col[ic]:
                nc.gpsimd.tensor_tensor(Rp[:], U[:], U[:] if first["p"] else Rp[:],
                                        op=OP.max)
                first["p"] = False
            else:
                nc.vector.tensor_tensor(Rd[:], U[:], U[:] if first["d"] else Rd[:],
                                        op=OP.max)
                first["d"] = False

        nc.sync.dma_start(out=ra_d[s], in_=rsa[:])
        nc.sync.dma_start(out=rb_d[s], in_=rmb[:])
        nc.sync.dma_start(out=rd_d[s], in_=Rd[:])
        if have_p:
            nc.sync.dma_start(out=rp_d[s], in_=Rp[:])
        else:
            nc.sync.dma_start(out=rp_d[s], in_=Rd[:])


def _fp8_split3(v):
    """v (f32 array) -> three e4m3 planes summing to ~v (rel err ~2e-4)."""
    c1 = v.astype(NP_FP8)
    r1 = v - c1.astype(np.float32)
    c2 = r1.astype(NP_FP8)
    r2 = r1 - c2.astype(np.float32)
    c3 = r2.astype(NP_FP8)
    return c1, c2, c3


def _make_aug(data, norm_half, miss, own_sign, L):
    """Build the [D, 2, L] fp8 augmented operand for one batch side.

    tile0 = data (fp8). tile1 rows encode the bilinear form so that
      W = x.y - x2/2 - y2/2 - 480*(1-m_i) - 480*(1-m_j).
    """
    out = np.zeros((D, 2, L), dtype=NP_FP8)
    out[:, 0, :] = data[:, :L].astype(NP_FP8)
    c1, c2, c3 = _fp8_split3(norm_half[:L])
    ones = np.ones(L, dtype=np.float32)
    pen = (miss[:L] * MASKPEN).astype(np.float32)
    if own_sign > 0:   # x side: [x2c1,x2c2,x2c3, 1,1,1, pen_i, 2]
        rows = [c1.astype(np.float32), c2.astype(np.float32),
                c3.astype(np.float32), ones, ones, ones, pen, 2.0 * ones]
    else:              # y side: [-1,-1,-1, -y2c1,-y2c2,-y2c3, -2, -pen_j]
        rows = [-ones, -ones, -ones,
                -c1.astype(np.float32), -c2.astype(np.float32),
                -c3.astype(np.float32), -2.0 * ones, -pen]
    for k, r in enumerate(rows):
        out[k, 1, :] = r.astype(NP_FP8)
    return out


def prepare_in_maps(x, y, mask):
    """Returns (in_maps, crops, assign): 8 per-core input dicts; slot crops;
    assign[c][s] = original batch index handled by core c slot s."""
    x = np.asarray(x, dtype=np.float32)
    y = np.asarray(y, dtype=np.float32)
    m = np.asarray(mask).astype(np.float32)
    last = np.array([int(np.max(np.nonzero(m[b])[0])) + 1 if m[b].any() else 1
                     for b in range(B)])
    order = np.argsort(-last, kind="stable")
    crops = []
    for s in range(BPC):
        ranks = order[s * CORES:(s + 1) * CORES]
        L = int(np.max(last[ranks]))
        L = min(N, ((L + ICH - 1) // ICH) * ICH)
        crops.append(max(ICH, L))
    x2h = 0.5 * (x * x).sum(axis=1)   # [B, N]
    y2h = 0.5 * (y * y).sum(axis=1)
    in_maps = []
    assign = []
    for c in range(CORES):
        im = {}
        slots = []
        for s in range(BPC):
            b = int(order[s * CORES + c])
            slots.append(b)
            L = crops[s]
            miss = 1.0 - m[b]
            im[f"xa{s}"] = _make_aug(x[b], x2h[b], miss, +1, L)
            im[f"ya{s}"] = _make_aug(y[b], y2h[b], miss, -1, L)
        in_maps.append(im)
        assign.append(slots)
    return in_maps, crops, assign


def finish(core_outs, crops, assign, m):
    """core_outs[c]: ra/rb{s} [128, nic] f32; rd/rp{s} [128, L] bf16."""
    m = np.asarray(m).astype(np.float64)
    total = 0.0
    tiny = 1e-300
    for c in range(CORES):
        for s, L in enumerate(crops):
            b = assign[c][s]
            nic = L // ICH
            wA, _ = slot_geometry(L)
            mb = m[b]
            ra = np.asarray(core_outs[c][f"ra{s}"], dtype=np.float64)
            rb = np.asarray(core_outs[c][f"rb{s}"], dtype=np.float64)
            rd = np.asarray(core_outs[c][f"rd{s}"], dtype=np.float64)
            rp = np.asarray(core_outs[c][f"rp{s}"], dtype=np.float64)
            # per-row W-max: LSE over the exp half, exact over the raw half
            md = np.maximum(np.log(np.maximum(ra, tiny)) - LSE_BIAS, rb)
            mrow = mb[:L].reshape(nic, ICH).T   # [128, nic] mask
            # per-col W-max from the chains (exp-space left, raw right)
            ch = np.maximum(rd, rp).max(axis=0)
            cmax = np.empty(L)
            cmax[:wA] = np.log(np.maximum(ch[:wA], tiny)) - LSE_BIAS
            cmax[wA:] = ch[wA:]
            total += (md * mrow).sum() + (cmax * mb[:L]).sum()
    return np.float32(-2.0 * total / B)


_NC = None
_NC_CROPS = None


def kernel(x, y, mask):
    global _NC, _NC_CROPS
    in_maps, crops, assign = prepare_in_maps(x, y, mask)
    key = tuple(crops)
    if _NC is None or _NC_CROPS != key:
        _NC = build_nc(crops)
        _NC_CROPS = key
    from concourse.bass_utils import run_bass_kernel_spmd
    res = run_bass_kernel_spmd(_NC, in_maps, list(range(CORES)))
    return finish([res.results[c] for c in range(CORES)], crops, assign,
                  np.asarray(mask))


# revision 19
# speedup vs baseline: 1.0582x; 1.0582x over previous
"""Chamfer loss Bass/Tile kernel for Trainium2 (8 NeuronCores, SPMD).

Problem: x, y [B=32, D=128, N=2048] f32, mask [B, N] bool (shared by x and y).
  d[b,i,j] = ||x_i - y_j||^2;  loss = mean_b( sum_j min_i d + sum_i min_j d )
  (mins/sums over valid entries only).

Strategy (v5):
  - ONE fp8 (e4m3) DoubleRow matmul per [128 x L] tile computes
      W = x.y - x2/2 - y2/2 - 480*(1-m_i) - 480*(1-m_j)  (= -d/2, biased)
    directly in PSUM: the DoubleRow second k-tile carries 8 augmented
    contraction rows encoding the norms (3-term fp8 residual splits, ~2e-4
    relative) and the mask penalties. 0.5 PE cycles/output element, no
    prefill, no downstream bias work.
  - Crop: mask is a prefix; only W[i<L, j<L] can matter, with L =
    ceil(last_set_bit/128)*128. Batches sorted by len across cores so the 4
    per-core slots share compile-time crops (one NEFF, SPMD).
  - Every chunk [128, L] of PSUM is consumed by a SPLIT evacuation, halving
    PSUM residency (the pipeline pacer) and balancing the engines:
      cols [0, WA):  ACT Exp-evacuates -> exp(W+22) bf16, its accumulator
        emits the row sum (softmin/LSE at beta=1 on the d/2 scale).
      cols [WA, L):  Pool tensor_scalar evacuates W bf16 with an exact
        rowmax accumulator.
    Both land in ONE U tile (exp-space left, raw right; columns are
    consistent across chunks). One full-width running tensor_tensor max
    per chunk builds the colmax chain: DVE mostly, Pool for a few chunks
    (two independent chains, combined on the host).
  - Device ships the two accumulator panels [128, nic] and the chain tiles
    [128, L] bf16; host does partition-maxes, logs, masks, -2/B (tiny numpy).
"""

import numpy as np
import ml_dtypes
from contextlib import ExitStack

import concourse.mybir as mybir
import concourse.tile as tile
from concourse import bacc

F32 = mybir.dt.float32
BF16 = mybir.dt.bfloat16
FP8 = mybir.dt.float8e4
AX = mybir.AxisListType
OP = mybir.AluOpType
ACTF = mybir.ActivationFunctionType
PM = mybir.MatmulPerfMode

B, D, N = 32, 128, 2048
CORES = 8
BPC = B // CORES          # batch slots per core
ICH = 128                 # i-chunk size (PSUM partition dim)
MASKPEN = 240.0           # TRN fp8e4m3 max normal; paired with a +/-2 partner
NP_FP8 = ml_dtypes.float8_e4m3   # concourse dt.py maps float8e4 to this
LSE_BIAS = 22.0           # global exp shift: exp(W + 22) spans ~[1e-33, 3e33]

WA_FRAC = 0.46            # ACT (exp) share of each chunk's columns
POOL_COL_FRAC = 0.0      # fraction of chunks whose colmax chain runs on Pool


def slot_geometry(L):
    """(wA, pool_col): ACT column split and per-chunk pool-colmax flags."""
    nic = L // ICH
    wA = int(round(WA_FRAC * L / 16)) * 16
    wA = min(max(wA, 16), L - 16)
    k = max(0, round(POOL_COL_FRAC * nic))
    pool_col = [False] * nic
    step = nic / k if k else 0
    for t in range(k):
        pool_col[min(nic - 1, int(t * step + step / 2))] = True
    return wA, pool_col


def build_nc(crops):
    nc = bacc.Bacc("TRN2", target_bir_lowering=False, debug=False)
    xa_d, ya_d, ra_d, rb_d, rd_d, rp_d = [], [], [], [], [], []
    for s, L in enumerate(crops):
        nic = L // ICH
        xa_d.append(nc.dram_tensor(f"xa{s}", [D, 2, L], FP8, kind="ExternalInput").ap())
        ya_d.append(nc.dram_tensor(f"ya{s}", [D, 2, L], FP8, kind="ExternalInput").ap())
        ra_d.append(nc.dram_tensor(f"ra{s}", [D, nic], F32, kind="ExternalOutput").ap())
        rb_d.append(nc.dram_tensor(f"rb{s}", [D, nic], F32, kind="ExternalOutput").ap())
        rd_d.append(nc.dram_tensor(f"rd{s}", [D, L], BF16, kind="ExternalOutput").ap())
        rp_d.append(nc.dram_tensor(f"rp{s}", [D, L], BF16, kind="ExternalOutput").ap())

    with tile.TileContext(nc) as tc:
        with ExitStack() as ctx:
            _emit(ctx, tc, crops, xa_d, ya_d, ra_d, rb_d, rd_d, rp_d)
    nc.compile()
    return nc


def _emit(ctx, tc, crops, xa_d, ya_d, ra_d, rb_d, rd_d, rp_d):
    nc = tc.nc
    io = ctx.enter_context(tc.tile_pool(name="io", bufs=2))
    up = ctx.enter_context(tc.tile_pool(name="up", bufs=4))
    rp_pool = ctx.enter_context(tc.tile_pool(name="rpool", bufs=2))
    small = ctx.enter_context(tc.tile_pool(name="small", bufs=2))
    pp = ctx.enter_context(tc.tile_pool(name="pp", bufs=2, space="PSUM"))
    pre = ctx.enter_context(tc.tile_pool(name="pre", bufs=1))

    lse_bias = pre.tile([D, 1], F32, tag="lse_bias")
    nc.gpsimd.memset(lse_bias[:], LSE_BIAS)
    # warm the Exp activation table before the first real chunk
    warm = pre.tile([D, 1], BF16, tag="warm")
    nc.scalar.activation(warm[:], lse_bias[:], ACTF.Exp, bias=0.0, scale=0.0)

    for s, L in enumerate(crops):
        nic = L // ICH
        wA, pool_col = slot_geometry(L)

        xa = io.tile([D, 2, L], FP8, tag="xa", name=f"xa{s}")
        ya = io.tile([D, 2, L], FP8, tag="ya", name=f"ya{s}")
        nc.sync.dma_start(out=ya[:, :, :512], in_=ya_d[s][:, :, :512])
        nc.sync.dma_start(out=xa[:, :, :ICH], in_=xa_d[s][:, :, :ICH])
        if L > 512:
            nc.sync.dma_start(out=ya[:, :, 512:], in_=ya_d[s][:, :, 512:])
        nc.sync.dma_start(out=xa[:, :, ICH:], in_=xa_d[s][:, :, ICH:])

        Rd = rp_pool.tile([D, L], BF16, tag="Rd", name=f"Rd{s}")
        Rp = rp_pool.tile([D, L], BF16, tag="Rp", name=f"Rp{s}")
        rsa = small.tile([D, nic], F32, tag="rsa", name=f"rsa{s}")  # sumexp
        rmb = small.tile([D, nic], F32, tag="rmb", name=f"rmb{s}")  # raw max
        first = {"d": True, "p": True}
        have_p = any(pool_col)

        for ic in range(nic):
            ps = pp.tile([D, L], F32, tag="ps")
            for j0 in range(0, L, 512):
                jw = min(512, L - j0)
                nc.tensor.matmul(
                    ps[:, j0:j0 + jw],
                    lhsT=xa[:, :, ic * ICH:(ic + 1) * ICH],
                    rhs=ya[:, :, j0:j0 + jw],
                    start=True, stop=True,
                    perf_mode=PM.DoubleRow)

            U = up.tile([D, L], BF16, tag="u")
            # split evacuation: exp-left on ACT (LSE row sums), raw-right on
            # Pool (exact row maxes); PSUM freed when both halves are read
            nc.scalar.activation(U[:, :wA], ps[:, :wA], ACTF.Exp,
                                 bias=lse_bias[:], scale=1.0,
                                 accum_out=rsa[:, ic:ic + 1])
            nc.gpsimd.tensor_scalar(U[:, wA:], ps[:, wA:], 0.0, None,
                                    op0=OP.add, op1=OP.max,
                                    accum_out=rmb[:, ic:ic + 1])
            if pool_col[ic]:
                nc.gpsimd.tensor_tensor(Rp[:], U[:], U[:] if first["p"] else Rp[:],
                                        op=OP.max)
                first["p"] = False
            else:
                nc.vector.tensor_tensor(Rd[:], U[:], U[:] if first["d"] else Rd[:],
                                        op=OP.max)
                first["d"] = False

        nc.sync.dma_start(out=ra_d[s], in_=rsa[:])
        nc.sync.dma_start(out=rb_d[s], in_=rmb[:])
        nc.sync.dma_start(out=rd_d[s], in_=Rd[:])
        if have_p:
            nc.sync.dma_start(out=rp_d[s], in_=Rp[:])
        else:
            nc.sync.dma_start(out=rp_d[s], in_=Rd[:])


def _fp8_split3(v):
    """v (f32 array) -> three e4m3 planes summing to ~v (rel err ~2e-4)."""
    c1 = v.astype(NP_FP8)
    r1 = v - c1.astype(np.float32)
    c2 = r1.astype(NP_FP8)
    r2 = r1 - c2.astype(np.float32)
    c3 = r2.astype(NP_FP8)
    return c1, c2, c3


def _make_aug(data, norm_half, miss, own_sign, L):
    """Build the [D, 2, L] fp8 augmented operand for one batch side.

    tile0 = data (fp8). tile1 rows encode the bilinear form so that
      W = x.y - x2/2 - y2/2 - 480*(1-m_i) - 480*(1-m_j).
    """
    out = np.zeros((D, 2, L), dtype=NP_FP8)
    out[:, 0, :] = data[:, :L].astype(NP_FP8)
    c1, c2, c3 = _fp8_split3(norm_half[:L])
    ones = np.ones(L, dtype=np.float32)
    pen = (miss[:L] * MASKPEN).astype(np.float32)
    if own_sign > 0:   # x side: [x2c1,x2c2,x2c3, 1,1,1, pen_i, 2]
        rows = [c1.astype(np.float32), c2.astype(np.float32),
                c3.astype(np.float32), ones, ones, ones, pen, 2.0 * ones]
    else:              # y side: [-1,-1,-1, -y2c1,-y2c2,-y2c3, -2, -pen_j]
        rows = [-ones, -ones, -ones,
                -c1.astype(np.float32), -c2.astype(np.float32),
                -c3.astype(np.float32), -2.0 * ones, -pen]
    for k, r in enumerate(rows):
        out[k, 1, :] = r.astype(NP_FP8)
    return out


def prepare_in_maps(x, y, mask):
    """Returns (in_maps, crops, assign): 8 per-core input dicts; slot crops;
    assign[c][s] = original batch index handled by core c slot s."""
    x = np.asarray(x, dtype=np.float32)
    y = np.asarray(y, dtype=np.float32)
    m = np.asarray(mask).astype(np.float32)
    last = np.array([int(np.max(np.nonzero(m[b])[0])) + 1 if m[b].any() else 1
                     for b in range(B)])
    order = np.argsort(-last, kind="stable")
    crops = []
    for s in range(BPC):
        ranks = order[s * CORES:(s + 1) * CORES]
        L = int(np.max(last[ranks]))
        L = min(N, ((L + ICH - 1) // ICH) * ICH)
        crops.append(max(ICH, L))
    x2h = 0.5 * (x * x).sum(axis=1)   # [B, N]
    y2h = 0.5 * (y * y).sum(axis=1)
    in_maps = []
    assign = []
    for c in range(CORES):
        im = {}
        slots = []
        for s in range(BPC):
            b = int(order[s * CORES + c])
            slots.append(b)
            L = crops[s]
            miss = 1.0 - m[b]
            im[f"xa{s}"] = _make_aug(x[b], x2h[b], miss, +1, L)
            im[f"ya{s}"] = _make_aug(y[b], y2h[b], miss, -1, L)
        in_maps.append(im)
        assign.append(slots)
    return in_maps, crops, assign


def finish(core_outs, crops, assign, m):
    """core_outs[c]: ra/rb{s} [128, nic] f32; rd/rp{s} [128, L] bf16."""
    m = np.asarray(m).astype(np.float64)
    total = 0.0
    tiny = 1e-300
    for c in range(CORES):
        for s, L in enumerate(crops):
            b = assign[c][s]
            nic = L // ICH
            wA, _ = slot_geometry(L)
            mb = m[b]
            ra = np.asarray(core_outs[c][f"ra{s}"], dtype=np.float64)
            rb = np.asarray(core_outs[c][f"rb{s}"], dtype=np.float64)
            rd = np.asarray(core_outs[c][f"rd{s}"], dtype=np.float64)
            rp = np.asarray(core_outs[c][f"rp{s}"], dtype=np.float64)
            # per-row W-max: LSE over the exp half, exact over the raw half
            md = np.maximum(np.log(np.maximum(ra, tiny)) - LSE_BIAS, rb)
            mrow = mb[:L].reshape(nic, ICH).T   # [128, nic] mask
            # per-col W-max from the chains (exp-space left, raw right)
            ch = np.maximum(rd, rp).max(axis=0)
            cmax = np.empty(L)
            cmax[:wA] = np.log(np.maximum(ch[:wA], tiny)) - LSE_BIAS
            cmax[wA:] = ch[wA:]
            total += (md * mrow).sum() + (cmax * mb[:L]).sum()
    return np.float32(-2.0 * total / B)


_NC = None
_NC_CROPS = None


def kernel(x, y, mask):
    global _NC, _NC_CROPS
    in_maps, crops, assign = prepare_in_maps(x, y, mask)
    key = tuple(crops)
    if _NC is None or _NC_CROPS != key:
        _NC = build_nc(crops)
        _NC_CROPS = key
    from concourse.bass_utils import run_bass_kernel_spmd
    res = run_bass_kernel_spmd(_NC, in_maps, list(range(CORES)))
    return finish([res.results[c] for c in range(CORES)], crops, assign,
                  np.asarray(mask))
